# revision 1
# baseline (speedup 1.0000x reference)
"""GATv2 3-layer GNN forward on 8 Trainium2 NeuronCores (Bass/Tile).

Sharding: edges (with self-loops) sorted by dst; core c owns dst nodes
[5000c, 5000(c+1)) so all segment reductions are core-local. Node tables
are replicated via per-layer AllGather.

Per 127-node block, edges run in 128-edge tiles (GB tiles batched for
the elementwise ops):
    psum_t = S_T^T @ xr_s_block + I @ gather(xl_s_table, src)  (TensorE)
      where xl_s/xr_s/we_s carry |att| prefolded into the feature columns
      and S_T's row 127 holds the per-edge ea value (folds the ea*we
      rank-1 term into the same matmul).
    logits = sum_c sign(att_c) * prelu(psum_t, 0.2)    (ScalarE + VectorE;
      positive scales commute with leaky-relu, so no abs/linear split)
    ex = exp(logits)            (softmax max-subtraction skipped: logits
                                 are O(1) for this model's weight scale)
    psum_blk += S^T @ [ex * xl_s | ex]                 (TensorE one-hot)
Block epilogue: h = tanh(psum_y * (1/den) * (1/|att|) + bias); next-layer
xl/xr via PE transpose + matmul; xl staged to DRAM and AllGathered.
Pooling: one-hot matmul on local graph ids, indirect-DMA scatter to
[512,8], AllReduce, then the tiny linear head.

Host work is index/layout preprocessing and weight fusion only (sort,
blocking, one-hot S_T tiles, |att| scaling); activations stay on device.
"""
import sys

for _p in ("/opt/trn_rl_repo",):
    if _p not in sys.path:
        sys.path.insert(0, _p)

import numpy as np

N = 40000
E = 500000
B = 512
NC = 8
NPC = N // NC            # nodes per core
BLK = 127                # real nodes per 128-row block (row 127 = ea/we slot)
NBLK = -(-NPC // BLK)    # blocks per core (40)
PADN = NBLK * 128        # padded node rows per core (5120)
HEADS = [(8, 32), (8, 16), (1, 8)]   # (H, C) per layer
DIMS = [h * c for h, c in HEADS]     # 256, 128, 8
WIDTHS = [d + h for d, (h, c) in zip(DIMS, HEADS)]  # scatter width: 264, 136, 9
GB = 4                   # edge tiles per elementwise batch
POOLPAD = 768

_CACHE = {}


def _padrow(n):
    c, nl = np.divmod(n, NPC)
    b, r = np.divmod(nl, BLK)
    return PADN * c + 128 * b + r


def _host_preprocess(x, edge_index, edge_attr, batch):
    src = np.asarray(edge_index[0], np.int64)
    dst = np.asarray(edge_index[1], np.int64)
    ea = np.asarray(edge_attr, np.float32).reshape(-1)

    # self loops, fill_value='mean' of incoming edge_attr
    deg = np.zeros(N, np.float32)
    np.add.at(deg, dst, np.float32(1.0))
    esum = np.zeros(N, np.float32)
    np.add.at(esum, dst, ea)
    loop_attr = np.where(deg > 0, esum / np.maximum(deg, 1.0), 0.0).astype(np.float32)
    src_f = np.concatenate([src, np.arange(N, dtype=np.int64)])
    dst_f = np.concatenate([dst, np.arange(N, dtype=np.int64)])
    ea_f = np.concatenate([ea, loop_attr]).astype(np.float32)

    order = np.argsort(dst_f, kind="stable")
    src_s, dst_s, ea_s = src_f[order], dst_f[order], ea_f[order]
    src_pad = _padrow(src_s).astype(np.int32)

    bounds = np.searchsorted(dst_s, np.arange(0, N + 1, 1))
    tiles_pb = []
    for b in range(NBLK):
        mx = 0
        for c in range(NC):
            lo = bounds[min(c * NPC + b * BLK, N)]
            hi = bounds[min(c * NPC + min((b + 1) * BLK, NPC), N)]
            mx = max(mx, hi - lo)
        tiles_pb.append(-(-mx // 128))
    T = sum(tiles_pb)

    MAXNT = max(tiles_pb)
    # block-packed S_T: per block, partition p holds its rows of all nt
    # tiles contiguously -> one line-rate DMA per block
    st_blk = np.zeros((NC, NBLK, 128, MAXNT * 128), np.float32)
    src_all = np.zeros((NC, T, 128), np.int32)
    dst_all = np.full((NC, T, 128), 200.0, np.float32)
    t0 = 0
    for b in range(NBLK):
        nt = tiles_pb[b]
        for c in range(NC):
            lo = bounds[min(c * NPC + b * BLK, N)]
            hi = bounds[min(c * NPC + min((b + 1) * BLK, NPC), N)]
            ne = hi - lo
            dl = (dst_s[lo:hi] - c * NPC - b * BLK).astype(np.int64)
            ti = np.arange(ne) // 128          # tile within block
            pi = np.arange(ne) % 128
            st_blk[c, b, dl, ti * 128 + pi] = 1.0
            st_blk[c, b, 127, ti * 128 + pi] = ea_s[lo:hi]
            src_all[c, t0 + ti, pi] = src_pad[lo:hi]
            dst_all[c, t0 + ti, pi] = dl.astype(np.float32)
        t0 += nt
    src_sb = np.ascontiguousarray(src_all.transpose(0, 2, 1))
    dst_sb = np.ascontiguousarray(dst_all.transpose(0, 2, 1))

    # pooling metadata
    batch = np.asarray(batch, np.int64)
    gbase = np.array([batch[c * NPC] for c in range(NC)], np.int64)
    batchloc = np.full((NC, 128, NBLK), 200.0, np.float32)
    for c in range(NC):
        bl = batch[c * NPC:(c + 1) * NPC] - gbase[c]
        assert bl.max() < 127, "graph span exceeds 127 per core"
        for b in range(NBLK):
            nn = min((b + 1) * BLK, NPC) - b * BLK
            batchloc[c, :nn, b] = bl[b * BLK: b * BLK + nn]
    g_rows = np.zeros((NC, 128, 1), np.int32)
    for c in range(NC):
        rows = gbase[c] + np.arange(128)
        junk = B + 64 + np.arange(128)
        g_rows[c, :, 0] = np.where(rows < B, rows, junk)
    cnt = np.bincount(batch, minlength=B).astype(np.float32)
    rcnt = (1.0 / np.maximum(cnt, 1.0)).astype(np.float32)

    return dict(tiles_pb=tiles_pb, T=T, MAXNT=MAXNT, st_blk=st_blk, src_sb=src_sb,
                dst_sb=dst_sb, batchloc=batchloc, g_rows=g_rows, rcnt=rcnt)


def _fuse_weights(wl, wr, we, att, H, C):
    """|att| folded into the main columns (prelu commutes with positive
    scales); sign(att) applied after prelu; 1/|att| recovers the scatter."""
    absatt = np.maximum(np.abs(att).reshape(-1), 1e-8).astype(np.float32)  # [D]
    WL = (wl * absatt[None, :]).astype(np.float32)
    WR = (wr * absatt[None, :]).astype(np.float32)
    WE = (we * absatt[None, :]).astype(np.float32)
    sgn = np.sign(att).reshape(-1).astype(np.float32)
    attrecip = (1.0 / absatt).astype(np.float32)
    return WL, WR, WE, sgn, attrecip


def _host_weights(inp):
    out = {}
    W = []
    for i, (H, C) in enumerate(HEADS, start=1):
        W.append(_fuse_weights(np.asarray(inp[f"wl{i}"], np.float32),
                               np.asarray(inp[f"wr{i}"], np.float32),
                               np.asarray(inp[f"we{i}"], np.float32),
                               np.asarray(inp[f"att{i}"], np.float32), H, C))
    # layer-1 input fusion: ext = [x0, xyz, 1]; h0 = ext @ M
    M = np.zeros((5, 7), np.float32)
    M[0, :4] = np.asarray(inp["w0"], np.float32)[0]
    M[1, 4] = M[2, 5] = M[3, 6] = 1.0
    M[4, :4] = np.asarray(inp["b0"], np.float32)
    out["WL1f"] = (M @ W[0][0]).astype(np.float32)      # [5, 256]
    out["WR1f"] = (M @ W[0][1]).astype(np.float32)
    for i in (2, 3):
        out[f"WL{i}"] = W[i - 1][0]
        out[f"WR{i}"] = W[i - 1][1]
    for i in (1, 2, 3):
        out[f"weaug{i}"] = np.tile(W[i - 1][2], (1, NBLK)).astype(np.float32)
        out[f"sgnB{i}"] = np.tile(W[i - 1][3][None, :], (128, 1))
        out[f"attrecip{i}"] = np.tile(W[i - 1][4][None, :], (128, 1))
        out[f"biasRep{i}"] = np.tile(np.asarray(inp[f"b{i}"], np.float32)[None, :],
                                     (128, 1))
    out["w4rep"] = np.tile(np.asarray(inp["w4"], np.float32)[:, 0][None, :], (128, 1))
    out["b4"] = float(np.asarray(inp["b4"], np.float32)[0])
    return out


def _build_x_inputs(x):
    x = np.asarray(x, np.float32)
    ext = np.concatenate([x[:, :1], x[:, 1:], np.ones((N, 1), np.float32)], 1)
    extp = np.zeros((NC * PADN, 5), np.float32)
    extp[_padrow(np.arange(N))] = ext
    xt6_full = np.ascontiguousarray(extp.T)
    xt6_own = np.ascontiguousarray(extp.reshape(NC, PADN, 5).transpose(0, 2, 1))
    return xt6_full, xt6_own


def _build_program(tiles_pb, T, variant=""):
    import contextlib
    import concourse.bass as bass
    import concourse.bacc as bacc
    import concourse.mybir as mybir
    import concourse.tile as tile

    dt = mybir.dt
    f32 = dt.float32
    bf16 = dt.bfloat16
    i32 = dt.int32
    Alu = mybir.AluOpType
    Act = mybir.ActivationFunctionType
    IOA = bass.IndirectOffsetOnAxis

    NOGATHER = "nogather" in variant
    NOCC = "nocc" in variant
    REP = 1
    for part in variant.split("_"):
        if part.startswith("rep"):
            REP = int(part[3:])

    nc = bacc.Bacc("TRN2", target_bir_lowering=False, debug=False, num_devices=NC)

    ein = {}
    def EIN(name, shape, d=f32):
        ein[name] = nc.dram_tensor(name, list(shape), d, kind="ExternalInput")
        return ein[name]

    MAXNT = -(-T // NBLK) + 8  # conservative; actual passed via in_maps shape
    st_blk_d = EIN("st_blk", [NBLK, 128, max(tiles_pb) * 128], bf16)
    src_sb_d = EIN("src_sb", [128, T], i32)
    dst_sb_d = EIN("dst_sb", [128, T], bf16)
    xt6_full_d = EIN("xt6_full", [5, NC * PADN])
    xt6_own_d = EIN("xt6_own", [5, PADN])
    WL1f_d = EIN("WL1f", [5, DIMS[0]])
    WR1f_d = EIN("WR1f", [5, DIMS[0]])
    WL2_d = EIN("WL2", [DIMS[0], DIMS[1]], bf16)
    WR2_d = EIN("WR2", [DIMS[0], DIMS[1]], bf16)
    WL3_d = EIN("WL3", [DIMS[1], DIMS[2]], bf16)
    WR3_d = EIN("WR3", [DIMS[1], DIMS[2]], bf16)
    weaug_d = [EIN(f"weaug{i}", [1, NBLK * DIMS[i - 1]], bf16) for i in (1, 2, 3)]
    sgnB_d = [EIN(f"sgnB{i}", [128, DIMS[i - 1]], bf16) for i in (1, 2, 3)]
    attrecip_d = [EIN(f"attrecip{i}", [128, DIMS[i - 1]]) for i in (1, 2, 3)]
    biasRep_d = [EIN(f"biasRep{i}", [128, DIMS[i - 1]]) for i in (1, 2, 3)]
    iota_d = EIN("iota_row", [128, 128], bf16)
    ident_d = EIN("ident", [128, 128], bf16)
    batchloc_d = EIN("batchloc", [128, NBLK], bf16)
    g_rows_d = EIN("g_rows", [128, 1], i32)
    rcnt_d = EIN("rcnt", [128, 4])
    w4rep_d = EIN("w4rep", [128, 8])
    b4_d = EIN("b4v", [128, 1])

    out_d = nc.dram_tensor("out", [B, 1], f32, kind="ExternalOutput")

    tables = [nc.dram_tensor(f"table{i}", [NC * PADN, DIMS[i - 1]], bf16)
              for i in (1, 2, 3)]
    stages = [nc.dram_tensor(f"stage{i}", [PADN, DIMS[i - 1]], bf16)
              for i in (2, 3)]
    pool_full = nc.dram_tensor("pool_full", [POOLPAD, 8], f32)
    pool_red = nc.dram_tensor("pool_red", [B, 8], f32)

    NTILE = NC * PADN // 128  # 320

    if variant == "null":
        with tile.TileContext(nc) as tc:
            with tc.tile_pool(name="sb", bufs=1) as sb:
                t = sb.tile([128, 8], bf16)
                nc.sync.dma_start(t[:], st_blk_d[0, :, 0:8])
                t2 = sb.tile([128, 1], f32)
                nc.vector.tensor_copy(t2[:], t[:, 0:1])
                nc.sync.dma_start(out_d[0:128, :], t2[:])
        nc.compile()
        return nc

    with tile.TileContext(nc) as tc:
        ctx = contextlib.ExitStack()
        with ctx:
            consts = ctx.enter_context(tc.tile_pool(name="consts", bufs=1))
            meta = ctx.enter_context(tc.tile_pool(name="meta", bufs=1))
            xrp = ctx.enter_context(tc.tile_pool(name="xrp", bufs=1))
            stp = ctx.enter_context(tc.tile_pool(name="stp", bufs=6))
            gp = ctx.enter_context(tc.tile_pool(name="gp", bufs=3))
            sp = ctx.enter_context(tc.tile_pool(name="sp", bufs=3))
            ep = ctx.enter_context(tc.tile_pool(name="ep", bufs=3))
            pst = ctx.enter_context(tc.tile_pool(name="psum_t", bufs=2, space="PSUM"))
            psb = ctx.enter_context(tc.tile_pool(name="psum_blk", bufs=2, space="PSUM"))
            pse = ctx.enter_context(tc.tile_pool(name="psum_epi", bufs=1, space="PSUM"))
            chp = ctx.enter_context(tc.tile_pool(name="chunk", bufs=2))

            def load_const(dram, shape, d=f32):
                t = consts.tile(list(shape), d, tag=dram.name + "_c")
                nc.sync.dma_start(t[:], dram[:])
                return t
            iota_t = load_const(iota_d, [128, 128], bf16)
            ident_t = load_const(ident_d, [128, 128], bf16)
            WL1f_t = load_const(WL1f_d, [5, DIMS[0]])
            WR1f_t = load_const(WR1f_d, [5, DIMS[0]])
            WL2_t = [consts.tile([128, DIMS[1]], bf16, tag=f"wl2_{k}", name=f"wl2_{k}")
                     for k in range(2)]
            WR2_t = [consts.tile([128, DIMS[1]], bf16, tag=f"wr2_{k}", name=f"wr2_{k}")
                     for k in range(2)]
            for k in range(2):
                nc.sync.dma_start(WL2_t[k][:], WL2_d[k * 128:(k + 1) * 128, :])
                nc.sync.dma_start(WR2_t[k][:], WR2_d[k * 128:(k + 1) * 128, :])
            WL3_t = load_const(WL3_d, [128, DIMS[2]], bf16)
            WR3_t = load_const(WR3_d, [128, DIMS[2]], bf16)
            sgnB_t = [load_const(sgnB_d[i], [128, DIMS[i]], bf16) for i in range(3)]
            attrecip_t = [load_const(attrecip_d[i], [128, DIMS[i]]) for i in range(3)]
            biasRep_t = [load_const(biasRep_d[i], [128, DIMS[i]]) for i in range(3)]
            batchloc_t = load_const(batchloc_d, [128, NBLK], bf16)
            g_rows_t = load_const(g_rows_d, [128, 1], i32)
            rcnt_t = load_const(rcnt_d, [128, 4])
            w4rep_t = load_const(w4rep_d, [128, 8])
            b4_t = load_const(b4_d, [128, 1])
            src_t = meta.tile([128, T], i32)
            nc.sync.dma_start(src_t[:], src_sb_d[:])
            dst_t = meta.tile([128, T], bf16)
            nc.sync.dma_start(dst_t[:], dst_sb_d[:])

            xr_t = [xrp.tile([128, NBLK * DIMS[i]], bf16, tag=f"xr{i}", name=f"xr{i}")
                    for i in range(3)]
            for i in range(3):
                nc.sync.dma_start(xr_t[i][127:128, :], weaug_d[i][:])

            rep_ctx = tc.For_i(0, REP, 1) if REP > 1 else contextlib.nullcontext()
            rep_ctx.__enter__()

            # ---- preamble: build full table1 + own xr1 (f32 math, bf16 out) ----
            CH = 16
            for ch in range(NTILE // CH):
                xchunk = chp.tile([5, CH * 128], f32, tag="xchunk")
                nc.sync.dma_start(xchunk[:],
                                  xt6_full_d[:, ch * CH * 128:(ch + 1) * CH * 128])
                for j in range(CH):
                    pt = pse.tile([128, DIMS[0]], f32, tag="epi_ps", space="PSUM")
                    nc.tensor.matmul(pt[:], lhsT=xchunk[:, j * 128:(j + 1) * 128],
                                     rhs=WL1f_t[:], start=True, stop=True)
                    st = ep.tile([128, DIMS[0]], bf16, tag="pre_sb")
                    nc.vector.tensor_copy(st[:], pt[:])
                    i = ch * CH + j
                    nc.sync.dma_start(tables[0][i * 128:(i + 1) * 128, :], st[:])
            for ch in range(-(-NBLK // CH)):
                j0, j1 = ch * CH, min((ch + 1) * CH, NBLK)
                xchunk = chp.tile([5, CH * 128], f32, tag="xchunk")
                nc.sync.dma_start(xchunk[:, :(j1 - j0) * 128],
                                  xt6_own_d[:, j0 * 128:j1 * 128])
                for j in range(j1 - j0):
                    b = j0 + j
                    pt = pse.tile([128, DIMS[0]], f32, tag="epi_ps", space="PSUM")
                    nc.tensor.matmul(pt[:], lhsT=xchunk[:, j * 128:(j + 1) * 128],
                                     rhs=WR1f_t[:], start=True, stop=True)
                    D0 = DIMS[0]
                    nc.vector.tensor_copy(xr_t[0][0:127, b * D0:(b + 1) * D0],
                                          pt[0:127, :])

            # ---- layers ----
            pool_ps = psb.tile([128, 8], f32, tag="pool_ps", space="PSUM", bufs=1)
            for li in range(3):
                H, C = HEADS[li]
                D = DIMS[li]
                W = WIDTHS[li]
                PSLOT = max(D, 8)
                table = tables[li]
                is_last = li == 2

                if li > 0:
                    if NOCC:
                        nc.sync.dma_start(table[0:PADN, :], stages[li - 1][:])
                    else:
                        nc.gpsimd.collective_compute(
                            "AllGather", Alu.bypass,
                            replica_groups=[list(range(NC))],
                            ins=[stages[li - 1].ap().opt()],
                            outs=[table.ap().opt()],
                        )

                t0 = 0
                for b in range(NBLK):
                    nt = tiles_pb[b]
                    pblk = psb.tile([128, W], f32, tag="blk_ps", space="PSUM")
                    sblk = stp.tile([128, max(tiles_pb) * 128], bf16, tag="st_blk",
                                    bufs=2)
                    nc.scalar.dma_start(sblk[:, :nt * 128],
                                        st_blk_d[b, :, :nt * 128])
                    for g0 in range(0, nt, GB):
                        gs = min(GB, nt - g0)
                        ptile = pst.tile([128, GB, PSLOT], f32, tag="t_ps",
                                         space="PSUM")
                        gt = gp.tile([128, GB, D], bf16, tag="g_tile")
                        for i in range(gs):
                            t = t0 + g0 + i
                            if NOGATHER:
                                nc.sync.dma_start(
                                    gt[:, i, :],
                                    table[(t % 300) * 128:(t % 300 + 1) * 128, :])
                            else:
                                nc.gpsimd.indirect_dma_start(
                                    out=gt[:, i, :], out_offset=None, in_=table[:],
                                    in_offset=IOA(ap=src_t[:, t:t + 1], axis=0),
                                )
                            nc.tensor.matmul(
                                ptile[:, i, 0:D],
                                lhsT=sblk[:, (g0 + i) * 128:(g0 + i + 1) * 128],
                                rhs=xr_t[li][:, b * D:(b + 1) * D],
                                start=True, stop=False)
                            nc.tensor.matmul(ptile[:, i, 0:D], lhsT=ident_t[:],
                                             rhs=gt[:, i, :], start=False, stop=True)
                        # logits = sum_c sgn * prelu(t_s, 0.2); ex = exp(logits)
                        u = sp.tile([128, GB * D], bf16, tag="u_t")
                        nc.scalar.activation(
                            u[:, :gs * D].rearrange("p (g d) -> p g d", g=gs),
                            ptile[:, 0:gs, 0:D], Act.Prelu, alpha=0.2)
                        v = sp.tile([128, GB * D], bf16, tag="v_t")
                        nc.vector.tensor_tensor(
                            out=v[:, :gs * D].rearrange("p (g d) -> p g d", g=gs),
                            in0=u[:, :gs * D].rearrange("p (g d) -> p g d", g=gs),
                            in1=sgnB_t[li][:].unsqueeze(1).to_broadcast([128, gs, D]),
                            op=Alu.mult)
                        lg = sp.tile([128, GB * H], f32, tag="lg")
                        nc.vector.tensor_reduce(
                            out=lg[:, :gs * H].rearrange("p (g h) -> p g h", g=gs),
                            in_=v[:, :gs * D].rearrange("p (g h c) -> p g h c",
                                                        g=gs, h=H),
                            axis=mybir.AxisListType.X, op=Alu.add)
                        ex = sp.tile([128, GB * H], bf16, tag="ex")
                        nc.scalar.activation(ex[:, :gs * H], lg[:, :gs * H], Act.Exp)
                        yt = gp.tile([128, GB, W], bf16, tag="y_tile")
                        nc.vector.tensor_tensor(
                            out=yt[:, 0:gs, 0:D].rearrange("p g (h c) -> p g h c",
                                                           h=H),
                            in0=gt[:, 0:gs, :].rearrange("p g (h c) -> p g h c", h=H),
                            in1=ex[:, :gs * H].rearrange("p (g h) -> p g h", g=gs)
                                .unsqueeze(3).to_broadcast([128, gs, H, C]),
                            op=Alu.mult)
                        nc.vector.tensor_copy(
                            yt[:, 0:gs, D:W],
                            ex[:, :gs * H].rearrange("p (g h) -> p g h", g=gs))
                        Smat = stp.tile([128, GB * 128], bf16, tag="s_tile")
                        nc.vector.tensor_tensor(
                            out=Smat[:, :gs * 128].rearrange("p (g n) -> p g n", g=gs),
                            in0=dst_t[:, t0 + g0:t0 + g0 + gs].unsqueeze(2)
                                .to_broadcast([128, gs, 128]),
                            in1=iota_t[:].unsqueeze(1).to_broadcast([128, gs, 128]),
                            op=Alu.is_equal)
                        for i in range(gs):
                            nc.tensor.matmul(
                                pblk[:], lhsT=Smat[:, i * 128:(i + 1) * 128],
                                rhs=yt[:, i, :],
                                start=(g0 == 0 and i == 0),
                                stop=(g0 + i == nt - 1))
                    t0 += nt

                    # ---- block epilogue ----
                    den = sp.tile([128, H], f32, tag="den")
                    nc.vector.tensor_scalar_add(den[:], pblk[:, D:W], 1e-30)
                    rden = sp.tile([128, H], f32, tag="rden")
                    nc.vector.reciprocal(rden[:], den[:])
                    hr = ep.tile([128, D], f32, tag="hr")
                    nc.vector.tensor_tensor(
                        out=hr[:].rearrange("p (h c) -> p h c", h=H),
                        in0=pblk[:, 0:D].rearrange("p (h c) -> p h c", h=H),
                        in1=rden[:].unsqueeze(2).to_broadcast([128, H, C]),
                        op=Alu.mult)
                    nc.vector.tensor_tensor(out=hr[:], in0=hr[:],
                                            in1=attrecip_t[li][:], op=Alu.mult)
                    nc.vector.tensor_tensor(out=hr[:], in0=hr[:],
                                            in1=biasRep_t[li][:], op=Alu.add)
                    h = ep.tile([128, D], bf16, tag="h_blk")
                    nc.scalar.activation(h[:], hr[:], Act.Tanh)

                    if not is_last:
                        D2 = DIMS[li + 1]
                        WLn = [WL2_t[0], WL2_t[1]] if li == 0 else [WL3_t]
                        WRn = [WR2_t[0], WR2_t[1]] if li == 0 else [WR3_t]
                        nk = D // 128
                        hT = []
                        for k in range(nk):
                            tp = pse.tile([128, 128], bf16, tag="epi_ps", space="PSUM")
                            nc.tensor.transpose(tp[:], h[:, k * 128:(k + 1) * 128],
                                                ident_t[:])
                            hTk = ep.tile([128, 128], bf16, tag=f"hT{k}")
                            nc.vector.tensor_copy(hTk[:], tp[:])
                            hT.append(hTk)
                        pxl = pse.tile([128, D2], f32, tag="epi_ps", space="PSUM")
                        for k in range(nk):
                            nc.tensor.matmul(pxl[:], lhsT=hT[k][:], rhs=WLn[k][:],
                                             start=(k == 0), stop=(k == nk - 1))
                        xlout = ep.tile([128, D2], bf16, tag="xlout")
                        nc.vector.tensor_copy(xlout[:], pxl[:])
                        nc.sync.dma_start(stages[li][b * 128:(b + 1) * 128, :],
                                          xlout[:])
                        pxr = pse.tile([128, D2], f32, tag="epi_ps", space="PSUM")
                        for k in range(nk):
                            nc.tensor.matmul(pxr[:], lhsT=hT[k][:], rhs=WRn[k][:],
                                             start=(k == 0), stop=(k == nk - 1))
                        nc.vector.tensor_copy(
                            xr_t[li + 1][0:127, b * D2:(b + 1) * D2], pxr[0:127, :])
                    else:
                        Sg = stp.tile([128, 128], bf16, tag="sg_tile")
                        nc.vector.tensor_tensor(
                            out=Sg[:],
                            in0=batchloc_t[:, b:b + 1].to_broadcast([128, 128]),
                            in1=iota_t[:], op=Alu.is_equal)
                        nc.tensor.matmul(pool_ps[:], lhsT=Sg[:], rhs=h[:],
                                         start=(b == 0), stop=(b == NBLK - 1))

            # ---- pooling + head ----
            pool_sb = ep.tile([128, 8], f32, tag="pool_sb")
            nc.vector.tensor_copy(pool_sb[:], pool_ps[:])
            zero8 = consts.tile([128, 8], f32, tag="zero8")
            nc.gpsimd.memset(zero8[:], 0.0)
            for i in range(POOLPAD // 128):
                nc.sync.dma_start(pool_full[i * 128:(i + 1) * 128, :], zero8[:])
            nc.gpsimd.indirect_dma_start(
                out=pool_full[:], out_offset=IOA(ap=g_rows_t[:, :1], axis=0),
                in_=pool_sb[:], in_offset=None)
            if NOCC:
                nc.sync.dma_start(pool_red[:], pool_full[0:B, :])
            else:
                nc.gpsimd.collective_compute(
                    "AllReduce", Alu.add, replica_groups=[list(range(NC))],
                    ins=[pool_full.ap()[0:B, :].opt()], outs=[pool_red.ap().opt()])
            for i in range(B // 128):
                pt = ep.tile([128, 8], f32, tag="head_in")
                nc.sync.dma_start(pt[:], pool_red[i * 128:(i + 1) * 128, :])
                pw = ep.tile([128, 8], f32, tag="head_w")
                nc.vector.tensor_tensor(out=pw[:], in0=pt[:], in1=w4rep_t[:],
                                        op=Alu.mult)
                hred = ep.tile([128, 1], f32, tag="head_red")
                nc.vector.tensor_reduce(out=hred[:], in_=pw[:],
                                        axis=mybir.AxisListType.X, op=Alu.add)
                nc.vector.tensor_tensor(out=hred[:], in0=hred[:],
                                        in1=rcnt_t[:, i:i + 1], op=Alu.mult)
                nc.vector.tensor_tensor(out=hred[:], in0=hred[:], in1=b4_t[:],
                                        op=Alu.add)
                nc.sync.dma_start(out_d[i * 128:(i + 1) * 128, :], hred[:])

            rep_ctx.__exit__(None, None, None)

    nc.compile()
    return nc


def _get_program(inputs):
    pre = _host_preprocess(inputs["x"], inputs["edge_index"], inputs["edge_attr"],
                           inputs["batch"])
    key = tuple(pre["tiles_pb"])
    if key not in _CACHE:
        _CACHE[key] = _build_program(pre["tiles_pb"], pre["T"])
    return _CACHE[key], pre


def _make_in_maps(inputs, pre):
    import ml_dtypes
    bf16 = ml_dtypes.bfloat16
    wts = _host_weights(inputs)
    xt6_full, xt6_own = _build_x_inputs(inputs["x"])
    iota = np.tile(np.arange(128, dtype=np.float32), (128, 1))
    ident = np.eye(128, dtype=np.float32)
    in_maps = []
    for c in range(NC):
        m = dict(
            st_blk=pre["st_blk"][c].astype(bf16), src_sb=pre["src_sb"][c],
            dst_sb=pre["dst_sb"][c].astype(bf16),
            xt6_full=xt6_full, xt6_own=xt6_own[c],
            WL1f=wts["WL1f"], WR1f=wts["WR1f"],
            WL2=wts["WL2"].astype(bf16), WR2=wts["WR2"].astype(bf16),
            WL3=wts["WL3"].astype(bf16), WR3=wts["WR3"].astype(bf16),
            iota_row=iota.astype(bf16), ident=ident.astype(bf16),
            batchloc=pre["batchloc"][c].astype(bf16), g_rows=pre["g_rows"][c],
            rcnt=np.ascontiguousarray(pre["rcnt"].reshape(4, 128).T),
            w4rep=wts["w4rep"], b4v=np.full((128, 1), wts["b4"], np.float32),
        )
        for i in (1, 2, 3):
            m[f"weaug{i}"] = wts[f"weaug{i}"].astype(bf16)
            m[f"sgnB{i}"] = wts[f"sgnB{i}"].astype(bf16)
            m[f"attrecip{i}"] = wts[f"attrecip{i}"]
            m[f"biasRep{i}"] = wts[f"biasRep{i}"]
        in_maps.append(m)
    return in_maps


def kernel(**inputs):
    from concourse.bass_utils import run_bass_kernel_spmd
    nc, pre = _get_program(inputs)
    in_maps = _make_in_maps(inputs, pre)
    res = run_bass_kernel_spmd(nc, in_maps, core_ids=list(range(NC)))
    return np.asarray(res.results[0]["out"], np.float32)

